# revision 4
# baseline (speedup 1.0000x reference)
"""Trainium2 Bass kernel for nn_DeconvLayer (causal IIR filter).

Math: the reference computes v[i] = x[i] + sum_j w[j] v[i-1-j] for i >= F
(v[i]=0 for i<F), which is a stable LTI IIR filter.  Its impulse response
h decays geometrically (|h[128]| ~ 1e-13 for this w), so the recurrence is
exactly (to fp32 precision) a causal FIR convolution with K=128 taps
applied to x with the first F columns zeroed:

    y[:, n] = sum_{k=0}^{127} h[k] * xz[:, n-k]

Implementation: Toeplitz matmuls on the TensorEngine.  Blocks of 128 time
steps are transposed (PE identity-matmul) so time lands on the partition
axis, then y^T(block c) = A0^T.T @ x^T(c) + A1^T.T @ x^T(c-1) accumulated
in PSUM, then transposed back.

Sharding: N=131072 split into 8 column slabs of 16384 (+128-col halo from
the left), all B=256 rows on every core.
"""

import os
import sys

import numpy as np

if "/opt/trn_rl_repo" not in sys.path:
    sys.path.insert(0, "/opt/trn_rl_repo")

B = 256
N = 131072
F = 8
K = 128          # FIR taps == block size
P = 128          # partitions / block size
NCORES = 8
CORE_COLS = N // NCORES       # 16384
TW = 2048                     # new columns processed per tile iteration
CH = TW // P                  # 16 chunks of 128 per iteration
NIT = CORE_COLS // TW         # 8 iterations per row group
RG = B // P                   # 2 row groups

# conv matmul dtype: "f32r" (fast, 1 cyc/row) or "f32" (exact, 4 cyc/row)
CONV_DTYPE = os.environ.get("KERNEL_CONV_DTYPE", "f32")
TRANS_DTYPE = os.environ.get("KERNEL_TRANS_DTYPE", "f32")

_CACHE = {}


def _impulse_response(w64):
    """128-tap impulse response of the IIR filter, in float64."""
    h = np.zeros(K, np.float64)
    h[0] = 1.0
    for n in range(1, K):
        acc = 0.0
        for j in range(min(F, n)):
            acc += w64[j] * h[n - 1 - j]
        h[n] = acc
    return h


def _toeplitz_mats(h):
    """A0T[t, i] = h[i-t] (i>=t); A1T[t, i] = h[128+i-t] (t>i).

    Stored so that matmul(out, lhsT=A0T, rhs=xT) gives
    out[i, j] = sum_t h[i-t] * xT[t, j].
    """
    a0t = np.zeros((P, P), np.float32)
    a1t = np.zeros((P, P), np.float32)
    for t in range(P):
        for i in range(P):
            if i >= t:
                a0t[t, i] = h[i - t]
            elif K + i - t < K:  # k = 128 + i - t in [1, 127]
                a1t[t, i] = h[K + i - t]
    return a0t, a1t


def _build_nc():
    from contextlib import ExitStack

    import concourse.mybir as mybir
    import concourse.tile as tile
    from concourse import bacc, masks

    f32 = mybir.dt.float32
    f32r = mybir.dt.float32r
    conv_dt = f32r if CONV_DTYPE == "f32r" else f32

    nc = bacc.Bacc(
        "TRN2", target_bir_lowering=False, debug=False, num_devices=NCORES
    )
    x_in = nc.dram_tensor("x_in", [B, P + CORE_COLS], f32, kind="ExternalInput")
    a0t_d = nc.dram_tensor("a0t", [P, P], f32, kind="ExternalInput")
    a1t_d = nc.dram_tensor("a1t", [P, P], f32, kind="ExternalInput")
    y_out = nc.dram_tensor("y_out", [B, CORE_COLS], f32, kind="ExternalOutput")

    with tile.TileContext(nc) as tc, ExitStack() as ctx:
        const = ctx.enter_context(tc.tile_pool(name="const", bufs=1))
        ident = const.tile([P, P], f32)
        masks.make_identity(nc, ident[:])
        a0t_s = const.tile([P, P], f32)
        nc.sync.dma_start(a0t_s[:], a0t_d[:, :])
        a1t_s = const.tile([P, P], f32)
        nc.sync.dma_start(a1t_s[:], a1t_d[:, :])

        xpool = ctx.enter_context(tc.tile_pool(name="x", bufs=3))
        tpool = ctx.enter_context(tc.tile_pool(name="t", bufs=2))
        ytpool = ctx.enter_context(tc.tile_pool(name="yt", bufs=2))
        ypool = ctx.enter_context(tc.tile_pool(name="y", bufs=3))
        ps_t = ctx.enter_context(tc.tile_pool(name="ps_t", bufs=2, space="PSUM"))
        ps_yt = ctx.enter_context(tc.tile_pool(name="ps_yt", bufs=2, space="PSUM"))
        ps_y = ctx.enter_context(tc.tile_pool(name="ps_y", bufs=2, space="PSUM"))

        copy_engines = [nc.vector, nc.scalar]
        cp_idx = 0

        def copy(out_ap, in_ap):
            nonlocal cp_idx
            eng = copy_engines[cp_idx % 2]
            cp_idx += 1
            if eng is nc.vector:
                eng.tensor_copy(out_ap, in_ap)
            else:
                eng.copy(out_ap, in_ap)

        for g in range(RG):
            for it in range(NIT):
                xt = xpool.tile([P, TW + P], f32)
                nc.sync.dma_start(
                    xt[:], x_in[g * P : (g + 1) * P, it * TW : it * TW + TW + P]
                )

                # transpose the 17 chunks (incl. leading halo chunk)
                tbuf = tpool.tile([P, TW + P], f32)
                for grp in range(5):
                    lo = grp * 4
                    hi = min(CH + 1, lo + 4)
                    pst = ps_t.tile([P, 512], f32, tag="pst")
                    for s in range(lo, hi):
                        nc.tensor.transpose(
                            pst[:, (s - lo) * P : (s - lo + 1) * P],
                            xt[:, s * P : (s + 1) * P],
                            ident[:],
                        )
                    copy(tbuf[:, lo * P : hi * P], pst[:, : (hi - lo) * P])

                # conv: yT[block s] = A0T.T @ xT[s] + A1T.T @ xT[s-1]
                ytbuf = ytpool.tile([P, TW], f32)
                for q in range(4):
                    psyt = ps_yt.tile([P, 512], f32, tag="psyt")
                    nc.tensor.matmul(
                        psyt[:],
                        a0t_s[:].bitcast(conv_dt),
                        tbuf[:, P + q * 512 : P + q * 512 + 512].bitcast(conv_dt),
                        start=True,
                        stop=False,
                    )
                    nc.tensor.matmul(
                        psyt[:],
                        a1t_s[:].bitcast(conv_dt),
                        tbuf[:, q * 512 : q * 512 + 512].bitcast(conv_dt),
                        start=False,
                        stop=True,
                    )
                    copy(ytbuf[:, q * 512 : (q + 1) * 512], psyt[:])

                # transpose back to natural layout
                ybuf = ypool.tile([P, TW], f32)
                for q in range(4):
                    psy = ps_y.tile([P, 512], f32, tag="psy")
                    for c in range(4):
                        nc.tensor.transpose(
                            psy[:, c * P : (c + 1) * P],
                            ytbuf[:, (q * 4 + c) * P : (q * 4 + c + 1) * P],
                            ident[:],
                        )
                    copy(ybuf[:, q * 512 : (q + 1) * 512], psy[:])

                nc.sync.dma_start(
                    y_out[g * P : (g + 1) * P, it * TW : (it + 1) * TW], ybuf[:]
                )
    nc.compile()
    return nc


def _get_nc():
    key = (CONV_DTYPE, TRANS_DTYPE)
    if key not in _CACHE:
        _CACHE[key] = _build_nc()
    return _CACHE[key]


LAST_RESULTS = None


def kernel(x, w=None, _trace=False, **_ignored):
    global LAST_RESULTS
    from concourse.bass_utils import run_bass_kernel_spmd

    x = np.asarray(x, dtype=np.float32)
    assert x.shape == (B, N)
    if w is None:
        import jax
        import jax.numpy as jnp

        key = jax.random.key(0)
        _, k2 = jax.random.split(key)
        w = np.asarray(
            jax.random.normal(k2, (F,), dtype=jnp.float32) * 0.05
        )
    w = np.asarray(w, dtype=np.float32)

    h = _impulse_response(w.astype(np.float64))
    a0t, a1t = _toeplitz_mats(h)

    # zero the first F input columns (v[i]=0 for i<F handling)
    xz = x.copy()
    xz[:, :F] = 0.0

    in_maps = []
    for c in range(NCORES):
        lo = c * CORE_COLS
        if c == 0:
            halo = np.zeros((B, P), np.float32)
        else:
            halo = xz[:, lo - P : lo]
        x_in = np.ascontiguousarray(
            np.concatenate([halo, xz[:, lo : lo + CORE_COLS]], axis=1)
        )
        in_maps.append({"x_in": x_in, "a0t": a0t, "a1t": a1t})

    nc = _get_nc()
    res = run_bass_kernel_spmd(
        nc, in_maps, core_ids=list(range(NCORES)), trace=_trace
    )
    LAST_RESULTS = res
    y = np.concatenate([r["y_out"] for r in res.results], axis=1)
    return np.ascontiguousarray(y.astype(np.float32))


if __name__ == "__main__":
    # smoke test with random data against a numpy reference
    rng = np.random.default_rng(0)
    x = rng.standard_normal((B, N), dtype=np.float32)
    w = (rng.standard_normal(F) * 0.05).astype(np.float32)
    y = kernel(x, w)
    print("kernel ran, y shape:", y.shape)


# revision 8
# speedup vs baseline: 1.5899x; 1.5899x over previous
"""Trainium2 Bass kernel for nn_DeconvLayer (causal IIR filter).

Math: the reference IIR v[i] = x[i] + sum_j w[j] v[i-1-j] (i >= F, else 0)
has a geometrically-decaying impulse response h (|h[128]| ~ 1e-13), so it
equals a 128-tap causal FIR applied to x with the first F columns zeroed:

    y[:, n] = sum_{k=0}^{127} h[k] * xz[:, n-k]

Split off the identity tap (h[0] = 1):  y = xz + h' * xz, so the main data
path is exact and the matmul only computes the ~0.3-magnitude correction.

Layout trick: the host uploads x TRANSPOSED ([N, B]), so time lands on the
partition axis with no on-device transposes.  Per 128-step block c:

    yT[c] = xT[c] + A0'^T.T @ xT[c] + A1^T.T @ xT[c-1]

where A0'[t, i] = h[i-t] (i > t, identity tap removed), A1[t, i] =
h[128+i-t] (t > i) are 128x128 Toeplitz matrices (PSUM-accumulated).
The host transposes the result back.

Sharding: N = 131072 split into 8 column slabs of 16384 (+128-step halo
from the left neighbor), all B = 256 rows on every core.
"""

import os
import sys

import numpy as np

if "/opt/trn_rl_repo" not in sys.path:
    sys.path.insert(0, "/opt/trn_rl_repo")

B = 256
N = 131072
F = 8
K = 128          # FIR taps == block size
P = 128          # partitions / block size
NCORES = 8
CORE_COLS = N // NCORES       # 16384 time steps per core
CPI = 8                       # chunks (128-step blocks) produced per iteration
NIT = CORE_COLS // (CPI * P)  # 16 iterations per core
FREE = B                      # free dim per chunk (batch rows)

# conv matmul dtype: "f32r" (fast, 1 cyc/row) or "f32" (exact, 4 cyc/row)
CONV_DTYPE = os.environ.get("KERNEL_CONV_DTYPE", "f32")

_CACHE = {}


def _impulse_response(w64):
    h = np.zeros(K, np.float64)
    h[0] = 1.0
    for n in range(1, K):
        acc = 0.0
        for j in range(min(F, n)):
            acc += w64[j] * h[n - 1 - j]
        h[n] = acc
    return h


def _toeplitz_mats(h):
    """A0'[t, i] = h[i-t] for i > t (identity tap removed);
    A1[t, i] = h[128+i-t] for t > i."""
    a0t = np.zeros((P, P), np.float32)
    a1t = np.zeros((P, P), np.float32)
    for t in range(P):
        for i in range(P):
            if i > t:
                a0t[t, i] = h[i - t]
            elif t > i:
                a1t[t, i] = h[K + i - t]
    return a0t, a1t


def _build_nc():
    from contextlib import ExitStack

    import concourse.mybir as mybir
    import concourse.tile as tile
    from concourse import bacc

    f32 = mybir.dt.float32
    f32r = mybir.dt.float32r
    conv_dt = f32r if CONV_DTYPE == "f32r" else f32

    nc = bacc.Bacc(
        "TRN2", target_bir_lowering=False, debug=False, num_devices=NCORES
    )
    # transposed input: row u = time step u - 128 (128-row halo prepended)
    x_in = nc.dram_tensor("x_in", [P + CORE_COLS, FREE], conv_dt, kind="ExternalInput")
    a0t_d = nc.dram_tensor("a0t", [P, P], conv_dt, kind="ExternalInput")
    a1t_d = nc.dram_tensor("a1t", [P, P], conv_dt, kind="ExternalInput")
    # transposed output
    y_out = nc.dram_tensor("y_out", [CORE_COLS, FREE], f32, kind="ExternalOutput")

    x_in3 = x_in.ap().rearrange("(c p) r -> p c r", p=P)   # [128, 129, 256]
    y_out3 = y_out.ap().rearrange("(c p) r -> p c r", p=P)  # [128, 128, 256]

    TW = CPI * FREE          # 2048 new free-columns per iteration
    TWH = (CPI + 1) * FREE   # with leading halo chunk

    with tile.TileContext(nc) as tc, ExitStack() as ctx:
        const = ctx.enter_context(tc.tile_pool(name="const", bufs=1))
        a0t_s = const.tile([P, P], conv_dt)
        nc.sync.dma_start(a0t_s[:], a0t_d[:, :])
        a1t_s = const.tile([P, P], conv_dt)
        nc.sync.dma_start(a1t_s[:], a1t_d[:, :])

        xpool = ctx.enter_context(tc.tile_pool(name="x", bufs=3))
        ypool = ctx.enter_context(tc.tile_pool(name="y", bufs=3))
        pspool = ctx.enter_context(tc.tile_pool(name="ps", bufs=4, space="PSUM"))

        for it in range(NIT):
            c0 = it * CPI  # first output chunk of this iteration
            xt = xpool.tile([P, CPI + 1, FREE], conv_dt)
            # input chunks c0-1 .. c0+7  (x_in chunk index = time chunk + 1)
            nc.sync.dma_start(xt[:], x_in3[:, c0 : c0 + CPI + 1, :])
            xt2 = xt[:].rearrange("p c r -> p (c r)")

            ybuf = ypool.tile([P, TW], f32)
            for q in range(4):
                ps = pspool.tile([P, 512], f32, tag="ps")
                nc.tensor.matmul(
                    ps[:],
                    a0t_s[:],
                    xt2[:, FREE + q * 512 : FREE + q * 512 + 512],
                    start=True,
                    stop=False,
                )
                nc.tensor.matmul(
                    ps[:],
                    a1t_s[:],
                    xt2[:, q * 512 : q * 512 + 512],
                    start=False,
                    stop=True,
                )
                # y = x + correction  (exact fp32 main path; DVE only —
                # ScalarE/GpSimd can't do tensor+tensor with a PSUM operand)
                nc.vector.tensor_add(
                    ybuf[:, q * 512 : (q + 1) * 512],
                    ps[:],
                    xt2[:, FREE + q * 512 : FREE + q * 512 + 512].bitcast(f32),
                )

            nc.sync.dma_start(
                y_out3[:, c0 : c0 + CPI, :],
                ybuf[:].rearrange("p (c r) -> p c r", c=CPI),
            )
    nc.compile()
    return nc


def _get_nc():
    key = CONV_DTYPE
    if key not in _CACHE:
        _CACHE[key] = _build_nc()
    return _CACHE[key]


LAST_RESULTS = None


def kernel(x, w=None, _trace=False, **_ignored):
    global LAST_RESULTS
    from concourse.bass_utils import run_bass_kernel_spmd

    x = np.asarray(x, dtype=np.float32)
    assert x.shape == (B, N)
    if w is None:
        import jax
        import jax.numpy as jnp

        key = jax.random.key(0)
        _, k2 = jax.random.split(key)
        w = np.asarray(jax.random.normal(k2, (F,), dtype=jnp.float32) * 0.05)
    w = np.asarray(w, dtype=np.float32)

    h = _impulse_response(w.astype(np.float64))
    a0t, a1t = _toeplitz_mats(h)

    # transposed, halo-padded input: row u = time step u - 128
    xt_pad = np.zeros((P + N, B), np.float32)
    xt_pad[P:] = x.T
    xt_pad[P : P + F] = 0.0  # v[i] = 0 for i < F

    in_maps = []
    for c in range(NCORES):
        lo = c * CORE_COLS
        x_in = xt_pad[lo : lo + P + CORE_COLS]  # contiguous view
        in_maps.append({"x_in": x_in, "a0t": a0t, "a1t": a1t})

    nc = _get_nc()
    res = run_bass_kernel_spmd(
        nc, in_maps, core_ids=list(range(NCORES)), trace=_trace
    )
    LAST_RESULTS = res
    yt = np.concatenate([r["y_out"] for r in res.results], axis=0)  # [N, B]
    return np.ascontiguousarray(yt.T)


if __name__ == "__main__":
    rng = np.random.default_rng(0)
    x = rng.standard_normal((B, N), dtype=np.float32)
    w = (rng.standard_normal(F) * 0.05).astype(np.float32)
    y = kernel(x, w)
    print("kernel ran, y shape:", y.shape)


# revision 9
# speedup vs baseline: 1.7155x; 1.0790x over previous
"""Trainium2 Bass kernel for nn_DeconvLayer (causal IIR filter).

Math: the reference IIR v[i] = x[i] + sum_j w[j] v[i-1-j] (i >= F, else 0)
has a geometrically-decaying impulse response h (|h[128]| ~ 1e-13), so it
equals a 128-tap causal FIR applied to x with the first F columns zeroed:

    y[:, n] = sum_{k=0}^{127} h[k] * xz[:, n-k]

Implemented as block-Toeplitz matmuls on the TensorEngine:

    yT[c] = A0^T.T @ xT[c] + A1^T.T @ xT[c-1]

with A0[t,i] = h[i-t] (i >= t, incl. the exact 1.0 identity tap) and
A1[t,i] = h[128+i-t] (t > i), PSUM-accumulated.

Precision trick: x is split on the host into fp16 hi + lo (x = hi + lo to
~2^-22 relative), and A into fp16 Ahi + Alo.  Three fp16 matmul streams
(Ahi@hi + Ahi@lo + Alo@hi) give ~fp32 accuracy at full PE rate (fp16 runs
1 cycle/row vs 4 for fp32), with the same DMA traffic as fp32 input.

Layout trick: the host uploads x transposed AND 128-blocked as
[t, chunk, r] so time lands on the partition axis with no on-device
transposes and every DMA partition-line is one contiguous read.

Sharding: N = 131072 split into 8 column slabs of 16384 (+128-step halo
from the left neighbor), all B = 256 rows on every core.
"""

import os
import sys

import numpy as np

if "/opt/trn_rl_repo" not in sys.path:
    sys.path.insert(0, "/opt/trn_rl_repo")

B = 256
N = 131072
F = 8
K = 128          # FIR taps == block size
P = 128          # partitions / block size
NCORES = 8
CORE_COLS = N // NCORES       # 16384 time steps per core
NCHUNK = CORE_COLS // P       # 128 chunks per core
CPI = 8                       # chunks produced per iteration
NIT = NCHUNK // CPI           # 16 iterations per core
FREE = B                      # free dim per chunk (batch rows)

_CACHE = {}


def _impulse_response(w64):
    h = np.zeros(K, np.float64)
    h[0] = 1.0
    for n in range(1, K):
        acc = 0.0
        for j in range(min(F, n)):
            acc += w64[j] * h[n - 1 - j]
        h[n] = acc
    return h


def _toeplitz_mats(h):
    """A0[t, i] = h[i-t] for i >= t (incl. identity tap);
    A1[t, i] = h[128+i-t] for t > i.  Returned in float64."""
    a0 = np.zeros((P, P), np.float64)
    a1 = np.zeros((P, P), np.float64)
    for t in range(P):
        for i in range(P):
            if i >= t:
                a0[t, i] = h[i - t]
            elif t > i:
                a1[t, i] = h[K + i - t]
    return a0, a1


def _split16(a64):
    hi = a64.astype(np.float16)
    lo = (a64 - hi.astype(np.float64)).astype(np.float16)
    return hi, lo


def _build_nc():
    from contextlib import ExitStack

    import concourse.mybir as mybir
    import concourse.tile as tile
    from concourse import bacc

    f32 = mybir.dt.float32
    f16 = mybir.dt.float16

    nc = bacc.Bacc(
        "TRN2", target_bir_lowering=False, debug=False, num_devices=NCORES
    )
    # blocked transposed input: [t, chunk, r] flattened to [128, (NCHUNK+1)*FREE]
    # chunk slot 0 is the halo (previous core's last 128 steps / zeros)
    W_IN = (NCHUNK + 1) * FREE
    xhi_d = nc.dram_tensor("x_hi", [P, W_IN], f16, kind="ExternalInput")
    xlo_d = nc.dram_tensor("x_lo", [P, W_IN], f16, kind="ExternalInput")
    a0hi_d = nc.dram_tensor("a0hi", [P, P], f16, kind="ExternalInput")
    a0lo_d = nc.dram_tensor("a0lo", [P, P], f16, kind="ExternalInput")
    a1hi_d = nc.dram_tensor("a1hi", [P, P], f16, kind="ExternalInput")
    a1lo_d = nc.dram_tensor("a1lo", [P, P], f16, kind="ExternalInput")
    # blocked transposed output [t, chunk, r]
    y_out = nc.dram_tensor("y_out", [P, NCHUNK * FREE], f32, kind="ExternalOutput")

    TWH = (CPI + 1) * FREE   # tile width incl leading halo chunk

    with tile.TileContext(nc) as tc, ExitStack() as ctx:
        const = ctx.enter_context(tc.tile_pool(name="const", bufs=1))
        a_tiles = {}
        for name, d in [
            ("a0hi", a0hi_d),
            ("a0lo", a0lo_d),
            ("a1hi", a1hi_d),
            ("a1lo", a1lo_d),
        ]:
            t = const.tile([P, P], f16, tag=name)
            nc.sync.dma_start(t[:], d[:, :])
            a_tiles[name] = t

        hpool = ctx.enter_context(tc.tile_pool(name="hi", bufs=3))
        lpool = ctx.enter_context(tc.tile_pool(name="lo", bufs=3))
        ypool = ctx.enter_context(tc.tile_pool(name="y", bufs=3))
        pspool = ctx.enter_context(tc.tile_pool(name="ps", bufs=4, space="PSUM"))

        for it in range(NIT):
            u0 = it * CPI * FREE  # start column (chunk slot it*CPI = halo of c0)
            hi = hpool.tile([P, TWH], f16)
            nc.sync.dma_start(hi[:], xhi_d[:, u0 : u0 + TWH])
            lo = lpool.tile([P, TWH], f16)
            nc.sync.dma_start(lo[:], xlo_d[:, u0 : u0 + TWH])

            ybuf = ypool.tile([P, CPI * FREE], f32)
            for q in range(4):
                ps = pspool.tile([P, 512], f32, tag="ps")
                cur = slice(FREE + q * 512, FREE + q * 512 + 512)
                prv = slice(q * 512, q * 512 + 512)
                nc.tensor.matmul(ps[:], a_tiles["a0hi"][:], hi[:, cur], start=True, stop=False)
                nc.tensor.matmul(ps[:], a_tiles["a0hi"][:], lo[:, cur], start=False, stop=False)
                nc.tensor.matmul(ps[:], a_tiles["a0lo"][:], hi[:, cur], start=False, stop=False)
                nc.tensor.matmul(ps[:], a_tiles["a1hi"][:], hi[:, prv], start=False, stop=False)
                nc.tensor.matmul(ps[:], a_tiles["a1hi"][:], lo[:, prv], start=False, stop=False)
                nc.tensor.matmul(ps[:], a_tiles["a1lo"][:], hi[:, prv], start=False, stop=True)
                # PSUM -> SBUF, alternating engines
                if q % 2 == 0:
                    nc.vector.tensor_copy(ybuf[:, q * 512 : (q + 1) * 512], ps[:])
                else:
                    nc.scalar.copy(ybuf[:, q * 512 : (q + 1) * 512], ps[:])

            nc.sync.dma_start(
                y_out[:, it * CPI * FREE : (it + 1) * CPI * FREE], ybuf[:]
            )
    nc.compile()
    return nc


def _get_nc():
    if "nc" not in _CACHE:
        _CACHE["nc"] = _build_nc()
    return _CACHE["nc"]


LAST_RESULTS = None


def kernel(x, w=None, _trace=False, **_ignored):
    global LAST_RESULTS
    from concourse.bass_utils import run_bass_kernel_spmd

    x = np.asarray(x, dtype=np.float32)
    assert x.shape == (B, N)
    if w is None:
        import jax
        import jax.numpy as jnp

        key = jax.random.key(0)
        _, k2 = jax.random.split(key)
        w = np.asarray(jax.random.normal(k2, (F,), dtype=jnp.float32) * 0.05)
    w = np.asarray(w, dtype=np.float32)

    h = _impulse_response(w.astype(np.float64))
    a0, a1 = _toeplitz_mats(h)
    a0hi, a0lo = _split16(a0)
    a1hi, a1lo = _split16(a1)

    # transposed, halo-padded, 128-blocked input: [t, chunk, r]
    xt_pad = np.zeros((P + N, B), np.float32)
    xt_pad[P:] = x.T
    xt_pad[P : P + F] = 0.0  # v[i] = 0 for i < F
    xb = np.ascontiguousarray(
        xt_pad.reshape(NCORES * NCHUNK + 1, P, B).transpose(1, 0, 2)
    )  # [128, 1025, 256] fp32
    xb_hi = xb.astype(np.float16)
    xb_lo = (xb - xb_hi.astype(np.float32)).astype(np.float16)

    in_maps = []
    for c in range(NCORES):
        lo_c = c * NCHUNK  # halo chunk slot for this core
        sl = np.s_[:, lo_c : lo_c + NCHUNK + 1, :]
        in_maps.append(
            {
                "x_hi": np.ascontiguousarray(xb_hi[sl]).reshape(P, -1),
                "x_lo": np.ascontiguousarray(xb_lo[sl]).reshape(P, -1),
                "a0hi": a0hi,
                "a0lo": a0lo,
                "a1hi": a1hi,
                "a1lo": a1lo,
            }
        )

    nc = _get_nc()
    res = run_bass_kernel_spmd(
        nc, in_maps, core_ids=list(range(NCORES)), trace=_trace
    )
    LAST_RESULTS = res
    # reassemble: per core [128, NCHUNK, FREE] -> [NCHUNK*P, FREE]
    parts = []
    for r in res.results:
        yb = r["y_out"].reshape(P, NCHUNK, B).transpose(1, 0, 2)  # [chunk, t, r]
        parts.append(yb.reshape(CORE_COLS, B))
    yt = np.concatenate(parts, axis=0)  # [N, B]
    return np.ascontiguousarray(yt.T)


if __name__ == "__main__":
    rng = np.random.default_rng(0)
    x = rng.standard_normal((B, N), dtype=np.float32)
    w = (rng.standard_normal(F) * 0.05).astype(np.float32)
    y = kernel(x, w)
    print("kernel ran, y shape:", y.shape)


# revision 13
# speedup vs baseline: 1.7514x; 1.0209x over previous
"""Trainium2 Bass kernel for nn_DeconvLayer (causal IIR filter).

Math: the reference IIR v[i] = x[i] + sum_j w[j] v[i-1-j] (i >= F, else 0)
has a geometrically-decaying impulse response h (|h[128]| ~ 1e-13), so it
equals a 128-tap causal FIR applied to x with the first F columns zeroed:

    y[:, n] = sum_{k=0}^{127} h[k] * xz[:, n-k]

Implemented as block-Toeplitz matmuls on the TensorEngine:

    yT[c] = A0^T.T @ xT[c] + A1^T.T @ xT[c-1]

with A0[t,i] = h[i-t] (i >= t, incl. the exact 1.0 identity tap) and
A1[t,i] = h[128+i-t] (t > i), PSUM-accumulated.

Precision trick: x is split on the host into fp16 hi + lo (x = hi + lo to
~2^-22 relative), and A into fp16 Ahi + Alo.  Three fp16 matmul streams
(Ahi@hi + Ahi@lo + Alo@hi) give ~fp32 accuracy at full PE rate (fp16 runs
1 cycle/row vs 4 for fp32), with the same DMA traffic as fp32 input.

Layout trick: the host uploads x transposed AND 128-blocked as
[t, chunk, r] so time lands on the partition axis with no on-device
transposes and every DMA partition-line is one contiguous read.

Sharding: N = 131072 split into 8 column slabs of 16384 (+128-step halo
from the left neighbor), all B = 256 rows on every core.
"""

import os
import sys

import numpy as np

if "/opt/trn_rl_repo" not in sys.path:
    sys.path.insert(0, "/opt/trn_rl_repo")

B = 256
N = 131072
F = 8
K = 128          # FIR taps == block size
P = 128          # partitions / block size
NCORES = 8
CORE_COLS = N // NCORES       # 16384 time steps per core
NCHUNK = CORE_COLS // P       # 128 chunks per core
CPI = 16                      # chunks produced per iteration
NIT = NCHUNK // CPI           # 8 iterations per core
FREE = B                      # free dim per chunk (batch rows)
QG = CPI * FREE // 512        # 512-wide PSUM groups per iteration (8)

_CACHE = {}


def _impulse_response(w64):
    h = np.zeros(K, np.float64)
    h[0] = 1.0
    for n in range(1, K):
        acc = 0.0
        for j in range(min(F, n)):
            acc += w64[j] * h[n - 1 - j]
        h[n] = acc
    return h


def _toeplitz_mats(h):
    """A0[t, i] = h[i-t] for i >= t (incl. identity tap);
    A1[t, i] = h[128+i-t] for t > i.  Returned in float64."""
    a0 = np.zeros((P, P), np.float64)
    a1 = np.zeros((P, P), np.float64)
    for t in range(P):
        for i in range(P):
            if i >= t:
                a0[t, i] = h[i - t]
            elif t > i:
                a1[t, i] = h[K + i - t]
    return a0, a1


def _split16(a64):
    hi = a64.astype(np.float16)
    lo = (a64 - hi.astype(np.float64)).astype(np.float16)
    return hi, lo


def _build_nc():
    from contextlib import ExitStack

    import concourse.mybir as mybir
    import concourse.tile as tile
    from concourse import bacc

    f32 = mybir.dt.float32
    f16 = mybir.dt.float16

    nc = bacc.Bacc(
        "TRN2",
        target_bir_lowering=False,
        debug=False,
        enable_asserts=False,
        num_devices=NCORES,
    )
    # blocked transposed input: [t, chunk, r] flattened to [128, (NCHUNK+1)*FREE]
    # chunk slot 0 is the halo (previous core's last 128 steps / zeros)
    W_IN = (NCHUNK + 1) * FREE
    xhi_d = nc.dram_tensor("x_hi", [P, W_IN], f16, kind="ExternalInput")
    xlo_d = nc.dram_tensor("x_lo", [P, W_IN], f16, kind="ExternalInput")
    a0hi_d = nc.dram_tensor("a0hi", [P, P], f16, kind="ExternalInput")
    a0lo_d = nc.dram_tensor("a0lo", [P, P], f16, kind="ExternalInput")
    a1hi_d = nc.dram_tensor("a1hi", [P, P], f16, kind="ExternalInput")
    a1lo_d = nc.dram_tensor("a1lo", [P, P], f16, kind="ExternalInput")
    # blocked transposed output [t, chunk, r]
    y_out = nc.dram_tensor("y_out", [P, NCHUNK * FREE], f32, kind="ExternalOutput")

    TWH = (CPI + 1) * FREE   # tile width incl leading halo chunk

    with tile.TileContext(nc) as tc, ExitStack() as ctx:
        const = ctx.enter_context(tc.tile_pool(name="const", bufs=1))
        a_tiles = {}
        for name, d in [
            ("a0hi", a0hi_d),
            ("a0lo", a0lo_d),
            ("a1hi", a1hi_d),
            ("a1lo", a1lo_d),
        ]:
            t = const.tile([P, P], f16, tag=name)
            nc.sync.dma_start(t[:], d[:, :])
            a_tiles[name] = t

        hpool = ctx.enter_context(tc.tile_pool(name="hi", bufs=3))
        lpool = ctx.enter_context(tc.tile_pool(name="lo", bufs=3))
        ypool = ctx.enter_context(tc.tile_pool(name="y", bufs=3))
        pspool = ctx.enter_context(tc.tile_pool(name="ps", bufs=8, space="PSUM"))

        # (stationary, moving, block-shift) per stream — stream-major order
        # so LDWEIGHTS happens once per stream per iteration, not per matmul
        STREAMS = [
            ("a0hi", "hi", 0),
            ("a0hi", "lo", 0),
            ("a0lo", "hi", 0),
            ("a1hi", "hi", 1),
            ("a1hi", "lo", 1),
            ("a1lo", "hi", 1),
        ]

        for it in range(NIT):
            u0 = it * CPI * FREE  # start column (chunk slot it*CPI = halo of c0)
            hi = hpool.tile([P, TWH], f16)
            nc.sync.dma_start(hi[:], xhi_d[:, u0 : u0 + TWH])
            lo = lpool.tile([P, TWH], f16)
            nc.sync.dma_start(lo[:], xlo_d[:, u0 : u0 + TWH])
            xin = {"hi": hi, "lo": lo}

            ybuf = ypool.tile([P, CPI * FREE], f32)
            pss = [
                pspool.tile([P, 512], f32, name=f"ps_{it}_{q}", tag="ps")
                for q in range(QG)
            ]
            for s, (a_name, x_name, shift) in enumerate(STREAMS):
                a_t = a_tiles[a_name]
                for q in range(QG):
                    off = (1 - shift) * FREE + q * 512
                    nc.tensor.matmul(
                        pss[q][:],
                        a_t[:],
                        xin[x_name][:, off : off + 512],
                        start=(s == 0),
                        stop=(s == len(STREAMS) - 1),
                    )
            for q in range(QG):
                # PSUM -> SBUF, alternating engines
                if q % 2 == 0:
                    nc.vector.tensor_copy(ybuf[:, q * 512 : (q + 1) * 512], pss[q][:])
                else:
                    nc.scalar.copy(ybuf[:, q * 512 : (q + 1) * 512], pss[q][:])

            nc.sync.dma_start(
                y_out[:, it * CPI * FREE : (it + 1) * CPI * FREE], ybuf[:]
            )
    nc.compile()
    return nc


def _get_nc():
    if "nc" not in _CACHE:
        _CACHE["nc"] = _build_nc()
    return _CACHE["nc"]


LAST_RESULTS = None


def kernel(x, w=None, _trace=False, **_ignored):
    global LAST_RESULTS
    from concourse.bass_utils import run_bass_kernel_spmd

    x = np.asarray(x, dtype=np.float32)
    assert x.shape == (B, N)
    if w is None:
        import jax
        import jax.numpy as jnp

        key = jax.random.key(0)
        _, k2 = jax.random.split(key)
        w = np.asarray(jax.random.normal(k2, (F,), dtype=jnp.float32) * 0.05)
    w = np.asarray(w, dtype=np.float32)

    h = _impulse_response(w.astype(np.float64))
    a0, a1 = _toeplitz_mats(h)
    a0hi, a0lo = _split16(a0)
    a1hi, a1lo = _split16(a1)

    # transposed, halo-padded, 128-blocked input: [t, chunk, r]
    xt_pad = np.zeros((P + N, B), np.float32)
    xt_pad[P:] = x.T
    xt_pad[P : P + F] = 0.0  # v[i] = 0 for i < F
    xb = np.ascontiguousarray(
        xt_pad.reshape(NCORES * NCHUNK + 1, P, B).transpose(1, 0, 2)
    )  # [128, 1025, 256] fp32
    xb_hi = xb.astype(np.float16)
    xb_lo = (xb - xb_hi.astype(np.float32)).astype(np.float16)

    in_maps = []
    for c in range(NCORES):
        lo_c = c * NCHUNK  # halo chunk slot for this core
        sl = np.s_[:, lo_c : lo_c + NCHUNK + 1, :]
        in_maps.append(
            {
                "x_hi": np.ascontiguousarray(xb_hi[sl]).reshape(P, -1),
                "x_lo": np.ascontiguousarray(xb_lo[sl]).reshape(P, -1),
                "a0hi": a0hi,
                "a0lo": a0lo,
                "a1hi": a1hi,
                "a1lo": a1lo,
            }
        )

    nc = _get_nc()
    res = run_bass_kernel_spmd(
        nc, in_maps, core_ids=list(range(NCORES)), trace=_trace
    )
    LAST_RESULTS = res
    # reassemble: per core [128, NCHUNK, FREE] -> [NCHUNK*P, FREE]
    parts = []
    for r in res.results:
        yb = r["y_out"].reshape(P, NCHUNK, B).transpose(1, 0, 2)  # [chunk, t, r]
        parts.append(yb.reshape(CORE_COLS, B))
    yt = np.concatenate(parts, axis=0)  # [N, B]
    return np.ascontiguousarray(yt.T)


if __name__ == "__main__":
    rng = np.random.default_rng(0)
    x = rng.standard_normal((B, N), dtype=np.float32)
    w = (rng.standard_normal(F) * 0.05).astype(np.float32)
    y = kernel(x, w)
    print("kernel ran, y shape:", y.shape)
